# revision 4
# baseline (speedup 1.0000x reference)
"""AsymQuantMatMul distributed Trainium2 kernel.

Full inputs: A [4,1024,4096] f32, B [4,1024,4096] f32.
Output: C [4,1024,1024] f32 with C[b] = dA[b] @ dB[b]^T where dA/dB are
per-batch-slice asymmetric-uint4 fake-quantized versions of A/B.

Sharding (8 cores): core c -> batch b=c//2, A-row half h=c%2.
Each core computes C[b][h*512:(h+1)*512, :].
Per-core inputs: a_own [512,4096] (matmul rows), a_oth [512,4096] (only for
the exact full-slice min/max of A[b]), b_in [1024,4096].

On-device math (exact vs reference up to reciprocal-boundary ties):
  q~ = clip(round(x/s), -z, 15-z)   (integers in [-15,15], exact in bf16)
  C  = (sA*sB) * (q~A @ q~B^T)      (bf16 matmul, fp32 PSUM — exact: products
                                     <=225, sums <= 4096*225 < 2^24)
round() uses the fp32 magic-constant trick: RNE(v + 12582912.0) - 12582912.0,
with the clip done in the shifted domain so it fuses into one tensor_scalar.
"""

import sys

import numpy as np

try:
    import concourse.bass as bass  # noqa: F401
except ImportError:
    sys.path.insert(0, "/opt/trn_rl_repo")

BS, H, W = 4, 1024, 4096
M = 512          # A rows per core
KT = W // 128    # 32 k-subtiles
RT_B = H // 128  # 8 B row-tiles
RT_A = M // 128  # 4 A row-tiles
MAGIC = 12582912.0  # 2^23 + 2^22: fp32 round-to-nearest-even shifter

_CACHE = {}


def _build():
    import concourse.bass as bass
    import concourse.mybir as mybir
    import concourse.tile as tile
    from concourse import bacc

    f32 = mybir.dt.float32
    bf16 = mybir.dt.bfloat16
    AX = mybir.AxisListType.X
    OP = mybir.AluOpType
    ACTF = mybir.ActivationFunctionType

    nc = bacc.Bacc("TRN2", target_bir_lowering=False, debug=False, num_devices=8)
    a_own = nc.declare_dram_parameter("a_own", [M, W], f32, isOutput=False)
    a_oth = nc.declare_dram_parameter("a_oth", [M, W], f32, isOutput=False)
    b_in = nc.declare_dram_parameter("b_in", [H, W], f32, isOutput=False)
    out = nc.declare_dram_parameter("out", [M, H], f32, isOutput=True)

    # DRAM bounce for cross-partition reduction of the 4 accumulators
    flip = nc.dram_tensor("flip", [4, 128], f32)

    a_own3 = a_own.rearrange("(r p) w -> r p w", p=128)
    a_oth3 = a_oth.rearrange("(r p) w -> r p w", p=128)
    b3 = b_in.rearrange("(r p) w -> r p w", p=128)
    out3 = out.rearrange("(r p) w -> r p w", p=128)

    with tile.TileContext(nc) as tc:
        with (
            tc.tile_pool(name="qat", bufs=1) as qat_pool,
            tc.tile_pool(name="qbt", bufs=1) as qbt_pool,
            tc.tile_pool(name="stage", bufs=2) as stage,
            tc.tile_pool(name="quant", bufs=2) as quant,
            tc.tile_pool(name="qnat", bufs=2) as qnat_pool,
            tc.tile_pool(name="small", bufs=1) as small,
            tc.tile_pool(name="outp", bufs=2) as outp,
            tc.tile_pool(name="psum", bufs=4, space="PSUM") as psum_pool,
            tc.tile_pool(name="psbc", bufs=1, space="PSUM") as psbc_pool,
        ):
            # Persistent quantized, transposed operands ([W-part, kt, rows])
            qAT = qat_pool.tile([128, KT, M], bf16)
            qBT = qbt_pool.tile([128, KT, H], bf16)

            # min/max accumulators, one column per streamed row-tile
            accs = {
                "amin": small.tile([128, RT_A * 2], f32, tag="amin", name="amin"),
                "amax": small.tile([128, RT_A * 2], f32, tag="amax", name="amax"),
                "bmin": small.tile([128, RT_B], f32, tag="bmin", name="bmin"),
                "bmax": small.tile([128, RT_B], f32, tag="bmax", name="bmax"),
            }

            # ---- pass 1: stream all data once, reduce min/max ----
            def p1(src3, rt, mincol, maxcol):
                t = stage.tile([128, W], f32, tag="stage")
                nc.gpsimd.dma_start(out=t[:], in_=src3[rt])
                nc.vector.tensor_reduce(out=mincol, in_=t[:], axis=AX, op=OP.min)
                nc.vector.tensor_reduce(out=maxcol, in_=t[:], axis=AX, op=OP.max)

            for rt in range(RT_B):
                p1(b3, rt, accs["bmin"][:, rt : rt + 1], accs["bmax"][:, rt : rt + 1])
            for rt in range(RT_A):
                p1(a_own3, rt, accs["amin"][:, rt : rt + 1], accs["amax"][:, rt : rt + 1])
            for rt in range(RT_A):
                p1(
                    a_oth3,
                    rt,
                    accs["amin"][:, RT_A + rt : RT_A + rt + 1],
                    accs["amax"][:, RT_A + rt : RT_A + rt + 1],
                )

            # ---- cross-partition reduce: [128,k] -> [128,1] -> DRAM -> [1,128] -> [1,1]
            red = small.tile([128, 4], f32, tag="red")
            for i, (name, op) in enumerate(
                [("amin", OP.min), ("amax", OP.max), ("bmin", OP.min), ("bmax", OP.max)]
            ):
                nc.vector.tensor_reduce(
                    out=red[:, i : i + 1], in_=accs[name][:], axis=AX, op=op
                )
                nc.gpsimd.dma_start(out=flip[i], in_=red[:, i : i + 1])

            flat = small.tile([1, 4 * 128], f32, tag="flat")
            for i in range(4):
                nc.gpsimd.dma_start(
                    out=flat[0:1, i * 128 : (i + 1) * 128], in_=flip[i][None, :]
                )

            # sc cols: 0 mnA, 1 mxA, 2 mnB, 3 mxB
            sc = small.tile([1, 16], f32, tag="sc")
            for i in range(4):
                nc.vector.tensor_reduce(
                    out=sc[0:1, i : i + 1],
                    in_=flat[0:1, i * 128 : (i + 1) * 128],
                    axis=AX,
                    op=(OP.min if i % 2 == 0 else OP.max),
                )

            # scalar math on partition 0; vals cols:
            # 0 inv_sA, 1 cAlo(=MAGIC-zA), 2 cAhi(=MAGIC+15-zA),
            # 3 inv_sB, 4 cBlo, 5 cBhi, 6 sA*sB
            vals = small.tile([1, 8], f32, tag="vals")
            tmp = small.tile([1, 8], f32, tag="tmpsc")

            def scalars(mn, mx, inv_col, clo_col, chi_col, s_col):
                # s = (mx-mn)/15 ; inv_s = 1/s
                nc.vector.tensor_tensor(out=tmp[0:1, 0:1], in0=mx, in1=mn, op=OP.subtract)
                nc.vector.tensor_scalar_mul(s_col, tmp[0:1, 0:1], 1.0 / 15.0)
                nc.vector.reciprocal(out=inv_col, in_=s_col)
                # zsh = clip(RNE(-mn/s + MAGIC), MAGIC, MAGIC+15)  (= MAGIC + z)
                nc.vector.tensor_scalar_mul(tmp[0:1, 1:2], inv_col, -1.0)
                nc.scalar.activation(
                    tmp[0:1, 2:3], mn, ACTF.Copy, bias=MAGIC, scale=tmp[0:1, 1:2]
                )
                nc.vector.tensor_scalar(
                    tmp[0:1, 3:4], tmp[0:1, 2:3], MAGIC + 15.0, MAGIC + 0.0, OP.min, OP.max
                )
                # c_lo = MAGIC - z = 2*MAGIC - zsh ; c_hi = c_lo + 15
                nc.vector.tensor_scalar(
                    clo_col, tmp[0:1, 3:4], -1.0, 2.0 * MAGIC, OP.mult, OP.add
                )
                nc.vector.tensor_scalar_add(chi_col, clo_col, 15.0)

            scalars(
                sc[0:1, 0:1], sc[0:1, 1:2],
                vals[0:1, 0:1], vals[0:1, 1:2], vals[0:1, 2:3], sc[0:1, 4:5],
            )
            scalars(
                sc[0:1, 2:3], sc[0:1, 3:4],
                vals[0:1, 3:4], vals[0:1, 4:5], vals[0:1, 5:6], sc[0:1, 5:6],
            )
            nc.vector.tensor_tensor(
                out=vals[0:1, 6:7], in0=sc[0:1, 4:5], in1=sc[0:1, 5:6], op=OP.mult
            )

            # broadcast [1,8] -> [128,8] via K=1 matmul with a ones column
            ones = small.tile([1, 128], f32, tag="ones")
            nc.vector.memset(ones[:], 1.0)
            ps_bc = psbc_pool.tile([128, 8], f32)
            nc.tensor.matmul(ps_bc[:], ones[:], vals[:], start=True, stop=True)
            bc = small.tile([128, 8], f32, tag="bc")
            nc.vector.tensor_copy(out=bc[:], in_=ps_bc[:])

            INV_A, CLO_A, CHI_A = bc[:, 0:1], bc[:, 1:2], bc[:, 2:3]
            INV_B, CLO_B, CHI_B = bc[:, 3:4], bc[:, 4:5], bc[:, 5:6]
            SASB = bc[:, 6:7]

            # ---- pass 2: re-stream, quantize, transpose ----
            def quantize(src3, rt, inv, clo, chi, qT, colbase):
                t = stage.tile([128, W], f32, tag="stage")
                nc.gpsimd.dma_start(out=t[:], in_=src3[rt])
                u = quant.tile([128, W], f32, tag="quant")
                # u = RNE(x*inv_s + MAGIC), then clip in shifted domain
                nc.scalar.activation(u[:], t[:], ACTF.Copy, bias=MAGIC, scale=inv)
                nc.vector.tensor_scalar(u[:], u[:], chi, clo, OP.min, OP.max)
                qn = qnat_pool.tile([128, W], bf16, tag="qnat")
                nc.vector.tensor_scalar_add(qn[:], u[:], -MAGIC)
                for kt in range(KT):
                    nc.sync.dma_start(
                        out=qT[:, kt, colbase : colbase + 128],
                        in_=qn[:, kt * 128 : (kt + 1) * 128],
                        transpose=True,
                    )

            for rt in range(RT_B):
                quantize(b3, rt, INV_B, CLO_B, CHI_B, qBT, rt * 128)
            for rt in range(RT_A):
                quantize(a_own3, rt, INV_A, CLO_A, CHI_A, qAT, rt * 128)

            # ---- pass 3: matmul + dequant epilogue ----
            for n in range(H // 512):
                for m in range(RT_A):
                    ps = psum_pool.tile([128, 512], f32)
                    for kt in range(KT):
                        nc.tensor.matmul(
                            ps[:],
                            qAT[:, kt, m * 128 : (m + 1) * 128],
                            qBT[:, kt, n * 512 : (n + 1) * 512],
                            start=(kt == 0),
                            stop=(kt == KT - 1),
                        )
                    o = outp.tile([128, 512], f32, tag="o")
                    nc.vector.tensor_scalar_mul(o[:], ps[:], SASB)
                    nc.gpsimd.dma_start(
                        out=out3[m, :, n * 512 : (n + 1) * 512], in_=o[:]
                    )

    nc.compile()
    return nc


def _get_nc():
    if "nc" not in _CACHE:
        _CACHE["nc"] = _build()
    return _CACHE["nc"]


def kernel(A: np.ndarray, B: np.ndarray) -> np.ndarray:
    from concourse.bass_utils import run_bass_kernel_spmd

    A = np.ascontiguousarray(A, dtype=np.float32)
    B = np.ascontiguousarray(B, dtype=np.float32)
    nc = _get_nc()

    in_maps = []
    for c in range(8):
        b, h = c // 2, c % 2
        in_maps.append(
            {
                "a_own": np.ascontiguousarray(A[b, h * M : (h + 1) * M]),
                "a_oth": np.ascontiguousarray(A[b, (1 - h) * M : (2 - h) * M]),
                "b_in": B[b],
            }
        )

    res = run_bass_kernel_spmd(nc, in_maps, core_ids=list(range(8)))
    C = np.empty((BS, H, H), dtype=np.float32)
    for c in range(8):
        b, h = c // 2, c % 2
        C[b, h * M : (h + 1) * M, :] = res.results[c]["out"]
    return C


# revision 11
# speedup vs baseline: 798.8330x; 798.8330x over previous
"""AsymQuantMatMul distributed Trainium2 kernel (v2: pair collectives).

Full inputs: A [4,1024,4096] f32, B [4,1024,4096] f32.
Output: C [4,1024,1024] f32 with C[b] = dA[b] @ dB[b]^T where dA/dB are
per-batch-slice asymmetric-uint4 fake-quantized versions of A/B.

Sharding (8 cores): core c -> batch b=c//2, half h=c%2.
Each core gets a_own = A[b] rows [h*512,(h+1)*512) and b_own = B[b] rows
[h*512,(h+1)*512) and computes C[b][h*512:(h+1)*512, :] (full columns).

Cross-core (pair) collectives:
  1. AllReduce(min) of packed (mnA, -mxA, mnB, -mxB) -> exact full-slice
     min/max for the quant scales.
  2. AllGather of the quantized+transposed B half (bf16, 4 MiB) so each
     core has the full q~B^T for its matmul columns.

On-device math (exact vs reference up to reciprocal-boundary ties):
  q~ = clip(round(x/s), -z, 15-z)   (integers in [-15,15], exact in bf16)
  C  = (sA*sB) * (q~A @ q~B^T)      (bf16 matmul, fp32 PSUM — exact)
round() uses the fp32 magic-constant trick: RNE(v + 12582912.0) - 12582912.0,
with the clip done in the shifted domain so it fuses into one tensor_scalar.
"""

import sys

import numpy as np

try:
    import concourse.bass as bass  # noqa: F401
except ImportError:
    sys.path.insert(0, "/opt/trn_rl_repo")

BS, H, W = 4, 1024, 4096
M = 512          # A rows per core
KT = W // 128    # 32 k-subtiles
RT = M // 128    # 4 row-tiles per half tensor
MAGIC = 12582912.0  # 2^23 + 2^22: fp32 round-to-nearest-even shifter

_CACHE = {}


def _build():
    import concourse.bass as bass
    import concourse.mybir as mybir
    import concourse.tile as tile
    from concourse import bacc

    f32 = mybir.dt.float32
    bf16 = mybir.dt.bfloat16
    AX = mybir.AxisListType.X
    XC = mybir.AxisListType.XYZWC
    OP = mybir.AluOpType
    ACTF = mybir.ActivationFunctionType
    PAIRS = [[0, 1], [2, 3], [4, 5], [6, 7]]

    nc = bacc.Bacc("TRN2", target_bir_lowering=False, debug=False, num_devices=8)
    a_own = nc.declare_dram_parameter("a_own", [M, W], f32, isOutput=False)
    b_own = nc.declare_dram_parameter("b_own", [M, W], f32, isOutput=False)
    out = nc.declare_dram_parameter("out", [M, H], f32, isOutput=True)

    a3 = a_own.rearrange("(r p) w -> r p w", p=128)
    b3 = b_own.rearrange("(r p) w -> r p w", p=128)
    out3 = out.rearrange("(r p) w -> r p w", p=128)

    with tile.TileContext(nc) as tc:
        with (
            tc.tile_pool(name="qat", bufs=1) as qat_pool,
            tc.tile_pool(name="qbt", bufs=1) as qbt_pool,
            tc.tile_pool(name="stage", bufs=2) as stage,
            tc.tile_pool(name="quant", bufs=1) as quant,
            tc.tile_pool(name="qnat", bufs=2) as qnat_pool,
            tc.tile_pool(name="small", bufs=1) as small,
            tc.tile_pool(name="outp", bufs=2) as outp,
            tc.tile_pool(name="psum", bufs=4, space="PSUM") as psum_pool,
            tc.tile_pool(name="psbc", bufs=1, space="PSUM") as psbc_pool,
            tc.tile_pool(name="dram", bufs=1, space="DRAM") as dram,
        ):
            # Persistent quantized, transposed operands ([W-part, kt, rows])
            qAT = qat_pool.tile([128, KT, M], bf16)
            qBTo = qat_pool.tile([128, KT, M], bf16, name="qBTo")  # own B half
            qBT = qbt_pool.tile([128, KT, H], bf16)  # full B after gather

            # min/max accumulators, one column per streamed row-tile
            accs = {
                "amin": small.tile([128, RT], f32, tag="amin", name="amin"),
                "amax": small.tile([128, RT], f32, tag="amax", name="amax"),
                "bmin": small.tile([128, RT], f32, tag="bmin", name="bmin"),
                "bmax": small.tile([128, RT], f32, tag="bmax", name="bmax"),
            }

            # ---- pass 1: stream own halves once, reduce min/max ----
            def p1(src3, rt, mincol, maxcol):
                t = stage.tile([128, W], f32, tag="stage")
                nc.sync.dma_start(out=t[:], in_=src3[rt])
                nc.vector.tensor_reduce(out=mincol, in_=t[:], axis=AX, op=OP.min)
                nc.vector.tensor_reduce(out=maxcol, in_=t[:], axis=AX, op=OP.max)

            for rt in range(RT):
                p1(b3, rt, accs["bmin"][:, rt : rt + 1], accs["bmax"][:, rt : rt + 1])
            for rt in range(RT):
                p1(a3, rt, accs["amin"][:, rt : rt + 1], accs["amax"][:, rt : rt + 1])

            # ---- cross-partition reduce. Cross-lane reduce and AllReduce use
            # max only, so pack negated mins: red cols = (-mnA, mxA, -mnB, mxB)
            red = small.tile([128, 4], f32, tag="red")
            nc.vector.tensor_reduce(out=red[:, 0:1], in_=accs["amin"][:], axis=AX, op=OP.min)
            nc.vector.tensor_scalar_mul(red[:, 0:1], red[:, 0:1], -1.0)
            nc.vector.tensor_reduce(out=red[:, 1:2], in_=accs["amax"][:], axis=AX, op=OP.max)
            nc.vector.tensor_reduce(out=red[:, 2:3], in_=accs["bmin"][:], axis=AX, op=OP.min)
            nc.vector.tensor_scalar_mul(red[:, 2:3], red[:, 2:3], -1.0)
            nc.vector.tensor_reduce(out=red[:, 3:4], in_=accs["bmax"][:], axis=AX, op=OP.max)

            scp = small.tile([1, 4], f32, tag="scp")
            for i in range(4):
                nc.gpsimd.tensor_reduce(
                    out=scp[0:1, i : i + 1], in_=red[:, i : i + 1], axis=XC, op=OP.max
                )

            cc_s_in = dram.tile([1, 4], f32, name="cc_s_in")
            cc_s_out = dram.tile([1, 4], f32, name="cc_s_out")
            nc.sync.dma_start(out=cc_s_in[:], in_=scp[:])
            nc.gpsimd.collective_compute(
                "AllReduce",
                OP.max,
                replica_groups=PAIRS,
                ins=[cc_s_in.opt()],
                outs=[cc_s_out.opt()],
            )
            # sc cols: 0 mnA, 1 mxA, 2 mnB, 3 mxB (un-negate cols 0,2)
            sc = small.tile([1, 16], f32, tag="sc")
            nc.sync.dma_start(out=sc[0:1, 0:4], in_=cc_s_out[:])
            nc.vector.tensor_scalar_mul(sc[0:1, 0:1], sc[0:1, 0:1], -1.0)
            nc.vector.tensor_scalar_mul(sc[0:1, 2:3], sc[0:1, 2:3], -1.0)

            # scalar math on partition 0; vals cols:
            # 0 inv_sA, 1 cAlo(=MAGIC-zA), 2 cAhi(=MAGIC+15-zA),
            # 3 inv_sB, 4 cBlo, 5 cBhi, 6 sA*sB
            vals = small.tile([1, 8], f32, tag="vals")
            tmp = small.tile([1, 8], f32, tag="tmpsc")

            def scalars(mn, mx, inv_col, clo_col, chi_col, s_col):
                # s = (mx-mn)/15 ; inv_s = 1/s
                nc.vector.tensor_tensor(out=tmp[0:1, 0:1], in0=mx, in1=mn, op=OP.subtract)
                nc.vector.tensor_scalar_mul(s_col, tmp[0:1, 0:1], 1.0 / 15.0)
                nc.vector.reciprocal(out=inv_col, in_=s_col)
                # zsh = clip(RNE(-mn/s + MAGIC), MAGIC, MAGIC+15)  (= MAGIC + z)
                nc.vector.tensor_scalar_mul(tmp[0:1, 1:2], inv_col, -1.0)
                nc.scalar.activation(
                    tmp[0:1, 2:3], mn, ACTF.Copy, bias=MAGIC, scale=tmp[0:1, 1:2]
                )
                nc.vector.tensor_scalar(
                    tmp[0:1, 3:4], tmp[0:1, 2:3], MAGIC + 15.0, MAGIC + 0.0, OP.min, OP.max
                )
                # c_lo = MAGIC - z = 2*MAGIC - zsh ; c_hi = c_lo + 15
                nc.vector.tensor_scalar(
                    clo_col, tmp[0:1, 3:4], -1.0, 2.0 * MAGIC, OP.mult, OP.add
                )
                nc.vector.tensor_scalar_add(chi_col, clo_col, 15.0)

            scalars(
                sc[0:1, 0:1], sc[0:1, 1:2],
                vals[0:1, 0:1], vals[0:1, 1:2], vals[0:1, 2:3], sc[0:1, 4:5],
            )
            scalars(
                sc[0:1, 2:3], sc[0:1, 3:4],
                vals[0:1, 3:4], vals[0:1, 4:5], vals[0:1, 5:6], sc[0:1, 5:6],
            )
            nc.vector.tensor_tensor(
                out=vals[0:1, 6:7], in0=sc[0:1, 4:5], in1=sc[0:1, 5:6], op=OP.mult
            )

            # broadcast [1,8] -> [128,8] via K=1 matmul with a ones column
            ones = small.tile([1, 128], f32, tag="ones")
            nc.vector.memset(ones[:], 1.0)
            ps_bc = psbc_pool.tile([128, 8], f32)
            nc.tensor.matmul(ps_bc[:], ones[:], vals[:], start=True, stop=True)
            bc = small.tile([128, 8], f32, tag="bc")
            nc.vector.tensor_copy(out=bc[:], in_=ps_bc[:])

            INV_A, CLO_A, CHI_A = bc[:, 0:1], bc[:, 1:2], bc[:, 2:3]
            INV_B, CLO_B, CHI_B = bc[:, 3:4], bc[:, 4:5], bc[:, 5:6]
            SASB = bc[:, 6:7]

            # ---- pass 2: re-stream, quantize, transpose ----
            def quantize(src3, rt, inv, clo, chi, qT, colbase):
                t = stage.tile([128, W], f32, tag="stage")
                nc.sync.dma_start(out=t[:], in_=src3[rt])
                u = quant.tile([128, W], f32, tag="quant")
                # u = RNE(x*inv_s + MAGIC), then clip in shifted domain
                nc.scalar.activation(u[:], t[:], ACTF.Copy, bias=MAGIC, scale=inv)
                nc.vector.tensor_scalar(u[:], u[:], chi, clo, OP.min, OP.max)
                qn = qnat_pool.tile([128, W], bf16, tag="qnat")
                nc.scalar.activation(qn[:], u[:], ACTF.Copy, bias=-MAGIC, scale=1.0)
                nc.sync.dma_start_transpose(
                    out=qT[:, :, colbase : colbase + 128], in_=qn[:]
                )

            for rt in range(RT):
                quantize(b3, rt, INV_B, CLO_B, CHI_B, qBTo, rt * 128)
            for rt in range(RT):
                quantize(a3, rt, INV_A, CLO_A, CHI_A, qAT, rt * 128)

            # ---- AllGather the transposed quantized B halves ----
            cc_g_in = dram.tile([128, KT, M], bf16, name="cc_g_in")
            cc_g_out = dram.tile([2, 128, KT, M], bf16, name="cc_g_out")
            nc.sync.dma_start(out=cc_g_in[:], in_=qBTo[:])
            nc.gpsimd.collective_compute(
                "AllGather",
                OP.bypass,
                replica_groups=PAIRS,
                ins=[cc_g_in.opt()],
                outs=[cc_g_out.opt()],
            )
            for j in range(2):
                nc.sync.dma_start(
                    out=qBT[:, :, j * M : (j + 1) * M], in_=cc_g_out[j]
                )

            # ---- pass 3: matmul + dequant epilogue ----
            for n in range(H // 512):
                for m in range(RT):
                    ps = psum_pool.tile([128, 512], f32)
                    for kt in range(KT):
                        nc.tensor.matmul(
                            ps[:],
                            qAT[:, kt, m * 128 : (m + 1) * 128],
                            qBT[:, kt, n * 512 : (n + 1) * 512],
                            start=(kt == 0),
                            stop=(kt == KT - 1),
                        )
                    o = outp.tile([128, 512], f32, tag="o")
                    nc.vector.tensor_scalar_mul(o[:], ps[:], SASB)
                    nc.sync.dma_start(
                        out=out3[m, :, n * 512 : (n + 1) * 512], in_=o[:]
                    )

    nc.compile()
    return nc


def _get_nc():
    if "nc" not in _CACHE:
        _CACHE["nc"] = _build()
    return _CACHE["nc"]


def kernel(A: np.ndarray, B: np.ndarray) -> np.ndarray:
    from concourse.bass_utils import run_bass_kernel_spmd

    A = np.ascontiguousarray(A, dtype=np.float32)
    B = np.ascontiguousarray(B, dtype=np.float32)
    nc = _get_nc()

    in_maps = []
    for c in range(8):
        b, h = c // 2, c % 2
        in_maps.append(
            {
                "a_own": np.ascontiguousarray(A[b, h * M : (h + 1) * M]),
                "b_own": np.ascontiguousarray(B[b, h * M : (h + 1) * M]),
            }
        )

    res = run_bass_kernel_spmd(nc, in_maps, core_ids=list(range(8)))
    C = np.empty((BS, H, H), dtype=np.float32)
    for c in range(8):
        b, h = c // 2, c % 2
        C[b, h * M : (h + 1) * M, :] = res.results[c]["out"]
    return C


# revision 13
# speedup vs baseline: 1432.5354x; 1.7933x over previous
"""AsymQuantMatMul distributed Trainium2 kernel (v3).

Full inputs: A [4,1024,4096] f32, B [4,1024,4096] f32.
Output: C [4,1024,1024] f32 with C[b] = dA[b] @ dB[b]^T where dA/dB are
per-batch-slice asymmetric-uint4 fake-quantized versions of A/B.

Sharding (8 cores): core c -> batch b=c//2, half h=c%2.
Per-core inputs: a_own = A[b] rows [h*512,(h+1)*512); b_rot = B[b] with the
core's own row-half FIRST (host rotates). The core computes
C[b][h-rows, rotated-cols]; the host un-rotates output columns.

Min/max for the quant scales: each core reduces a_own and the FIRST half of
b_rot (its own half); a 16-byte pair AllReduce(max) of (-mnA, mxA, -mnB, mxB)
yields exact full-slice min/max with no redundant streaming.

On-device math (exact vs reference up to reciprocal-boundary ties):
  q~ = clip(round(x/s), -z, 15-z)   (integers in [-15,15], exact in bf16)
  C  = (sA*sB) * (q~A @ q~B^T)      (bf16 matmul, fp32 PSUM — exact)
round() uses the fp32 magic-constant trick: RNE(v + 12582912.0) - 12582912.0,
with the clip done in the shifted domain so it fuses into one tensor_scalar.
"""

import sys

import numpy as np

try:
    import concourse.bass as bass  # noqa: F401
except ImportError:
    sys.path.insert(0, "/opt/trn_rl_repo")

BS, H, W = 4, 1024, 4096
M = 512          # A rows per core
KT = W // 128    # 32 k-subtiles
RT = M // 128    # 4 row-tiles per half
RT_B = H // 128  # 8 B row-tiles
MAGIC = 12582912.0  # 2^23 + 2^22: fp32 round-to-nearest-even shifter

_CACHE = {}


def _build():
    import concourse.bass as bass
    import concourse.mybir as mybir
    import concourse.tile as tile
    from concourse import bacc

    f32 = mybir.dt.float32
    bf16 = mybir.dt.bfloat16
    AX = mybir.AxisListType.X
    XC = mybir.AxisListType.XYZWC
    OP = mybir.AluOpType
    ACTF = mybir.ActivationFunctionType
    PAIRS = [[0, 1], [2, 3], [4, 5], [6, 7]]

    nc = bacc.Bacc("TRN2", target_bir_lowering=False, debug=False, num_devices=8)
    a_own = nc.declare_dram_parameter("a_own", [M, W], f32, isOutput=False)
    b_rot = nc.declare_dram_parameter("b_rot", [H, W], f32, isOutput=False)
    out = nc.declare_dram_parameter("out", [M, H], f32, isOutput=True)

    a3 = a_own.rearrange("(r p) w -> r p w", p=128)
    b3 = b_rot.rearrange("(r p) w -> r p w", p=128)
    out3 = out.rearrange("(r p) w -> r p w", p=128)

    with tile.TileContext(nc) as tc:
        with (
            tc.tile_pool(name="qat", bufs=1) as qat_pool,
            tc.tile_pool(name="qbt", bufs=1) as qbt_pool,
            tc.tile_pool(name="stage", bufs=3) as stage,
            tc.tile_pool(name="quant", bufs=2) as quant,
            tc.tile_pool(name="qnat", bufs=2) as qnat_pool,
            tc.tile_pool(name="small", bufs=1) as small,
            tc.tile_pool(name="outp", bufs=2) as outp,
            tc.tile_pool(name="psum", bufs=4, space="PSUM") as psum_pool,
            tc.tile_pool(name="psbc", bufs=1, space="PSUM") as psbc_pool,
            tc.tile_pool(name="dram", bufs=1, space="DRAM") as dram,
        ):
            # Persistent quantized, transposed operands ([W-part, kt, rows])
            qAT = qat_pool.tile([128, KT, M], bf16)
            qBT = qbt_pool.tile([128, KT, H], bf16)

            # min/max accumulators, one column per streamed row-tile
            accs = {
                "amin": small.tile([128, RT], f32, tag="amin", name="amin"),
                "amax": small.tile([128, RT], f32, tag="amax", name="amax"),
                "bmin": small.tile([128, RT], f32, tag="bmin", name="bmin"),
                "bmax": small.tile([128, RT], f32, tag="bmax", name="bmax"),
            }

            # ---- pass 1: stream own halves once, reduce min/max ----
            def p1(src3, rt, mincol, maxcol):
                t = stage.tile([128, W], f32, tag="stage")
                nc.sync.dma_start(out=t[:], in_=src3[rt])
                nc.vector.tensor_reduce(out=mincol, in_=t[:], axis=AX, op=OP.min)
                nc.vector.tensor_reduce(out=maxcol, in_=t[:], axis=AX, op=OP.max)

            for rt in range(RT):
                p1(b3, rt, accs["bmin"][:, rt : rt + 1], accs["bmax"][:, rt : rt + 1])
            for rt in range(RT):
                p1(a3, rt, accs["amin"][:, rt : rt + 1], accs["amax"][:, rt : rt + 1])

            # ---- cross-partition reduce (cross-lane + AllReduce support max
            # only: negate mins) -> neg-packed (-mnA, mxA, -mnB, mxB)
            red = small.tile([128, 4], f32, tag="red")
            nc.vector.tensor_reduce(out=red[:, 0:1], in_=accs["amin"][:], axis=AX, op=OP.min)
            nc.vector.tensor_scalar_mul(red[:, 0:1], red[:, 0:1], -1.0)
            nc.vector.tensor_reduce(out=red[:, 1:2], in_=accs["amax"][:], axis=AX, op=OP.max)
            nc.vector.tensor_reduce(out=red[:, 2:3], in_=accs["bmin"][:], axis=AX, op=OP.min)
            nc.vector.tensor_scalar_mul(red[:, 2:3], red[:, 2:3], -1.0)
            nc.vector.tensor_reduce(out=red[:, 3:4], in_=accs["bmax"][:], axis=AX, op=OP.max)

            scp = small.tile([1, 4], f32, tag="scp")
            for i in range(4):
                nc.gpsimd.tensor_reduce(
                    out=scp[0:1, i : i + 1], in_=red[:, i : i + 1], axis=XC, op=OP.max
                )

            cc_s_in = dram.tile([1, 4], f32, name="cc_s_in")
            cc_s_out = dram.tile([1, 4], f32, name="cc_s_out")
            nc.sync.dma_start(out=cc_s_in[:], in_=scp[:])
            nc.gpsimd.collective_compute(
                "AllReduce",
                OP.max,
                replica_groups=PAIRS,
                ins=[cc_s_in.opt()],
                outs=[cc_s_out.opt()],
            )
            # sc cols: 0 mnA, 1 mxA, 2 mnB, 3 mxB (un-negate cols 0,2)
            sc = small.tile([1, 16], f32, tag="sc")
            nc.sync.dma_start(out=sc[0:1, 0:4], in_=cc_s_out[:])
            nc.vector.tensor_scalar_mul(sc[0:1, 0:1], sc[0:1, 0:1], -1.0)
            nc.vector.tensor_scalar_mul(sc[0:1, 2:3], sc[0:1, 2:3], -1.0)

            # scalar math on partition 0; vals cols:
            # 0 inv_sA, 1 cAlo(=MAGIC-zA), 2 cAhi(=MAGIC+15-zA),
            # 3 inv_sB, 4 cBlo, 5 cBhi, 6 sA*sB
            vals = small.tile([1, 8], f32, tag="vals")
            tmp = small.tile([1, 8], f32, tag="tmpsc")

            def scalars(mn, mx, inv_col, clo_col, chi_col, s_col):
                # s = (mx-mn)/15 ; inv_s = 1/s
                nc.vector.tensor_tensor(out=tmp[0:1, 0:1], in0=mx, in1=mn, op=OP.subtract)
                nc.vector.tensor_scalar_mul(s_col, tmp[0:1, 0:1], 1.0 / 15.0)
                nc.vector.reciprocal(out=inv_col, in_=s_col)
                # zsh = clip(RNE(-mn/s + MAGIC), MAGIC, MAGIC+15)  (= MAGIC + z)
                nc.vector.tensor_scalar_mul(tmp[0:1, 1:2], inv_col, -1.0)
                nc.scalar.activation(
                    tmp[0:1, 2:3], mn, ACTF.Copy, bias=MAGIC, scale=tmp[0:1, 1:2]
                )
                nc.vector.tensor_scalar(
                    tmp[0:1, 3:4], tmp[0:1, 2:3], MAGIC + 15.0, MAGIC + 0.0, OP.min, OP.max
                )
                # c_lo = MAGIC - z = 2*MAGIC - zsh ; c_hi = c_lo + 15
                nc.vector.tensor_scalar(
                    clo_col, tmp[0:1, 3:4], -1.0, 2.0 * MAGIC, OP.mult, OP.add
                )
                nc.vector.tensor_scalar_add(chi_col, clo_col, 15.0)

            scalars(
                sc[0:1, 0:1], sc[0:1, 1:2],
                vals[0:1, 0:1], vals[0:1, 1:2], vals[0:1, 2:3], sc[0:1, 4:5],
            )
            scalars(
                sc[0:1, 2:3], sc[0:1, 3:4],
                vals[0:1, 3:4], vals[0:1, 4:5], vals[0:1, 5:6], sc[0:1, 5:6],
            )
            nc.vector.tensor_tensor(
                out=vals[0:1, 6:7], in0=sc[0:1, 4:5], in1=sc[0:1, 5:6], op=OP.mult
            )

            # broadcast [1,8] -> [128,8] via K=1 matmul with a ones column
            ones = small.tile([1, 128], f32, tag="ones")
            nc.vector.memset(ones[:], 1.0)
            ps_bc = psbc_pool.tile([128, 8], f32)
            nc.tensor.matmul(ps_bc[:], ones[:], vals[:], start=True, stop=True)
            bc = small.tile([128, 8], f32, tag="bc")
            nc.vector.tensor_copy(out=bc[:], in_=ps_bc[:])

            INV_A, CLO_A, CHI_A = bc[:, 0:1], bc[:, 1:2], bc[:, 2:3]
            INV_B, CLO_B, CHI_B = bc[:, 3:4], bc[:, 4:5], bc[:, 5:6]
            SASB = bc[:, 6:7]

            # ---- pass 2: re-stream, quantize, transpose ----
            def quantize(src3, rt, inv, clo, chi, qT, colbase, sub_eng):
                t = stage.tile([128, W], f32, tag="stage")
                nc.sync.dma_start(out=t[:], in_=src3[rt])
                u = quant.tile([128, W], f32, tag="quant")
                # u = RNE(x*inv_s + MAGIC), then clip in shifted domain
                nc.scalar.activation(u[:], t[:], ACTF.Copy, bias=MAGIC, scale=inv)
                nc.vector.tensor_scalar(u[:], u[:], chi, clo, OP.min, OP.max)
                qn = qnat_pool.tile([128, W], bf16, tag="qnat")
                if sub_eng == "act":
                    nc.scalar.activation(qn[:], u[:], ACTF.Copy, bias=-MAGIC, scale=1.0)
                else:
                    nc.gpsimd.tensor_scalar_add(qn[:], u[:], -MAGIC)
                nc.sync.dma_start_transpose(
                    out=qT[:, :, colbase : colbase + 128], in_=qn[:]
                )

            for rt in range(RT_B):
                quantize(b3, rt, INV_B, CLO_B, CHI_B, qBT, rt * 128,
                         "gpsimd" if rt % 2 else "act")
            for rt in range(RT):
                quantize(a3, rt, INV_A, CLO_A, CHI_A, qAT, rt * 128, "act")

            # ---- pass 3: matmul + dequant epilogue ----
            for n in range(H // 512):
                for m in range(RT):
                    ps = psum_pool.tile([128, 512], f32)
                    for kt in range(KT):
                        nc.tensor.matmul(
                            ps[:],
                            qAT[:, kt, m * 128 : (m + 1) * 128],
                            qBT[:, kt, n * 512 : (n + 1) * 512],
                            start=(kt == 0),
                            stop=(kt == KT - 1),
                        )
                    o = outp.tile([128, 512], f32, tag="o")
                    nc.vector.tensor_scalar_mul(o[:], ps[:], SASB)
                    nc.sync.dma_start(
                        out=out3[m, :, n * 512 : (n + 1) * 512], in_=o[:]
                    )

    nc.compile()
    return nc


def _get_nc():
    if "nc" not in _CACHE:
        _CACHE["nc"] = _build()
    return _CACHE["nc"]


def _in_maps(A, B):
    maps = []
    for c in range(8):
        b, h = c // 2, c % 2
        maps.append(
            {
                "a_own": np.ascontiguousarray(A[b, h * M : (h + 1) * M]),
                "b_rot": np.ascontiguousarray(
                    np.concatenate(
                        [B[b, h * M : (h + 1) * M], B[b, (1 - h) * M : (2 - h) * M]],
                        axis=0,
                    )
                ),
            }
        )
    return maps


def kernel(A: np.ndarray, B: np.ndarray) -> np.ndarray:
    from concourse.bass_utils import run_bass_kernel_spmd

    A = np.ascontiguousarray(A, dtype=np.float32)
    B = np.ascontiguousarray(B, dtype=np.float32)
    nc = _get_nc()

    res = run_bass_kernel_spmd(nc, _in_maps(A, B), core_ids=list(range(8)))
    C = np.empty((BS, H, H), dtype=np.float32)
    for c in range(8):
        b, h = c // 2, c % 2
        o = res.results[c]["out"]  # [512, 1024], columns in rotated order
        C[b, h * M : (h + 1) * M, h * M : (h + 1) * M] = o[:, 0:M]
        C[b, h * M : (h + 1) * M, (1 - h) * M : (2 - h) * M] = o[:, M : 2 * M]
    return C


# revision 14
# speedup vs baseline: 1577.7702x; 1.1014x over previous
"""AsymQuantMatMul distributed Trainium2 kernel (v4).

Full inputs: A [4,1024,4096] f32, B [4,1024,4096] f32.
Output: C [4,1024,1024] f32 with C[b] = dA[b] @ dB[b]^T where dA/dB are
per-batch-slice asymmetric-uint4 fake-quantized versions of A/B.

Sharding (8 cores): core c -> batch b=c//2, half h=c%2.
Per-core inputs: a_own = A[b] rows [h*512,(h+1)*512); b_rot = B[b] with the
core's own row-half FIRST (host rotates). The core computes
C[b][h-rows, rotated-cols]; the host un-rotates output columns.

Min/max for the quant scales: each core reduces a_own and the FIRST half of
b_rot (its own half); per-tensor 8-byte pair AllReduce(max) of (-mn, mx)
yields exact full-slice min/max. B's scale chain overlaps A's pass-1.

On-device math (exact vs reference up to reciprocal-boundary ties):
  q~ = clip(round(x/s), -z, 15-z)   (integers in [-15,15], exact in bf16)
  C  = (sA*sB) * (q~A @ q~B^T)      (bf16 matmul, fp32 PSUM — exact)
round() uses the fp32 magic-constant trick: RNE(v + 12582912.0) - 12582912.0,
with the clip done in the shifted domain so it fuses into one tensor_scalar.
"""

import sys

import numpy as np

try:
    import concourse.bass as bass  # noqa: F401
except ImportError:
    sys.path.insert(0, "/opt/trn_rl_repo")

BS, H, W = 4, 1024, 4096
M = 512          # A rows per core
KT = W // 128    # 32 k-subtiles
RT = M // 128    # 4 row-tiles per half
RT_B = H // 128  # 8 B row-tiles
MAGIC = 12582912.0  # 2^23 + 2^22: fp32 round-to-nearest-even shifter

_CACHE = {}


def _build():
    import concourse.bass as bass
    import concourse.bass_isa as bass_isa
    import concourse.mybir as mybir
    import concourse.tile as tile
    from concourse import bacc

    f32 = mybir.dt.float32
    bf16 = mybir.dt.bfloat16
    AX = mybir.AxisListType.X
    OP = mybir.AluOpType
    ACTF = mybir.ActivationFunctionType
    PAIRS = [[0, 1], [2, 3], [4, 5], [6, 7]]

    nc = bacc.Bacc("TRN2", target_bir_lowering=False, debug=False, num_devices=8)
    a_own = nc.declare_dram_parameter("a_own", [M, W], f32, isOutput=False)
    b_rot = nc.declare_dram_parameter("b_rot", [H, W], f32, isOutput=False)
    out = nc.declare_dram_parameter("out", [M, H], f32, isOutput=True)

    a3 = a_own.rearrange("(r p) w -> r p w", p=128)
    b3 = b_rot.rearrange("(r p) w -> r p w", p=128)
    out3 = out.rearrange("(r p) w -> r p w", p=128)

    with tile.TileContext(nc) as tc:
        with (
            tc.tile_pool(name="qat", bufs=1) as qat_pool,
            tc.tile_pool(name="qbt", bufs=1) as qbt_pool,
            tc.tile_pool(name="stage", bufs=3) as stage,
            tc.tile_pool(name="quant", bufs=2) as quant,
            tc.tile_pool(name="qnat", bufs=2) as qnat_pool,
            tc.tile_pool(name="small", bufs=1) as small,
            tc.tile_pool(name="outp", bufs=2) as outp,
            tc.tile_pool(name="psum", bufs=4, space="PSUM") as psum_pool,
            tc.tile_pool(name="dram", bufs=1, space="DRAM") as dram,
        ):
            # Persistent quantized, transposed operands ([W-part, kt, rows])
            qAT = qat_pool.tile([128, KT, M], bf16)
            qBT = qbt_pool.tile([128, KT, H], bf16)

            accs = {
                "amin": small.tile([128, RT], f32, tag="amin", name="amin"),
                "amax": small.tile([128, RT], f32, tag="amax", name="amax"),
                "bmin": small.tile([128, RT], f32, tag="bmin", name="bmin"),
                "bmax": small.tile([128, RT], f32, tag="bmax", name="bmax"),
            }

            def p1(src3, rt, mincol, maxcol):
                t = stage.tile([128, W], f32, tag="stage")
                nc.sync.dma_start(out=t[:], in_=src3[rt])
                nc.vector.tensor_reduce(out=mincol, in_=t[:], axis=AX, op=OP.min)
                nc.vector.tensor_reduce(out=maxcol, in_=t[:], axis=AX, op=OP.max)

            # vals cols per tensor X: 0 inv_sX, 1 cXlo(=MAGIC-zX),
            # 2 cXhi(=MAGIC+15-zX), 3 sX
            def scale_chain(pref, mincol_acc, maxcol_acc, vals):
                # neg-packed per-partition (-mn, mx), all-reduced across
                # partitions then across the pair; every partition uniform.
                red = small.tile([128, 2], f32, tag=f"red{pref}", name=f"red{pref}")
                nc.vector.tensor_reduce(out=red[:, 0:1], in_=mincol_acc, axis=AX, op=OP.min)
                nc.vector.tensor_scalar_mul(red[:, 0:1], red[:, 0:1], -1.0)
                nc.vector.tensor_reduce(out=red[:, 1:2], in_=maxcol_acc, axis=AX, op=OP.max)
                ar = small.tile([128, 2], f32, tag=f"ar{pref}", name=f"ar{pref}")
                nc.gpsimd.partition_all_reduce(
                    ar[:], red[:], channels=128, reduce_op=bass_isa.ReduceOp.max
                )
                cin = dram.tile([1, 2], f32, name=f"cin{pref}")
                cout = dram.tile([1, 2], f32, name=f"cout{pref}")
                nc.sync.dma_start(out=cin[:], in_=ar[0:1, :])
                nc.gpsimd.collective_compute(
                    "AllReduce", OP.max, replica_groups=PAIRS,
                    ins=[cin.opt()], outs=[cout.opt()],
                )
                g1 = small.tile([1, 2], f32, tag=f"g1{pref}", name=f"g1{pref}")
                nc.sync.dma_start(out=g1[:], in_=cout[:])
                g = small.tile([128, 2], f32, tag=f"g{pref}", name=f"g{pref}")
                nc.gpsimd.partition_broadcast(g[:], g1[:])
                mn = small.tile([128, 1], f32, tag=f"mn{pref}", name=f"mn{pref}")
                nc.vector.tensor_scalar_mul(mn[:], g[:, 0:1], -1.0)
                mx = g[:, 1:2]
                tmp = small.tile([128, 4], f32, tag=f"tmp{pref}", name=f"tmp{pref}")
                # s = (mx-mn)/15 ; inv_s = 1/s
                nc.vector.tensor_tensor(out=tmp[:, 0:1], in0=mx, in1=mn[:], op=OP.subtract)
                nc.vector.tensor_scalar_mul(vals[:, 3:4], tmp[:, 0:1], 1.0 / 15.0)
                nc.vector.reciprocal(out=vals[:, 0:1], in_=vals[:, 3:4])
                # zsh = clip(RNE(-mn/s + MAGIC), MAGIC, MAGIC+15)  (= MAGIC+z)
                nc.vector.tensor_scalar_mul(tmp[:, 1:2], vals[:, 0:1], -1.0)
                nc.scalar.activation(
                    tmp[:, 2:3], mn[:], ACTF.Copy, bias=MAGIC, scale=tmp[:, 1:2]
                )
                nc.vector.tensor_scalar(
                    tmp[:, 3:4], tmp[:, 2:3], MAGIC + 15.0, MAGIC + 0.0, OP.min, OP.max
                )
                # c_lo = MAGIC - z = 2*MAGIC - zsh ; c_hi = c_lo + 15
                nc.vector.tensor_scalar(
                    vals[:, 1:2], tmp[:, 3:4], -1.0, 2.0 * MAGIC, OP.mult, OP.add
                )
                nc.vector.tensor_scalar_add(vals[:, 2:3], vals[:, 1:2], 15.0)

            valsB = small.tile([128, 4], f32, tag="valsB", name="valsB")
            valsA = small.tile([128, 4], f32, tag="valsA", name="valsA")

            # pass 1 for B (own half), then B scale chain (overlaps A pass 1)
            for rt in range(RT):
                p1(b3, rt, accs["bmin"][:, rt : rt + 1], accs["bmax"][:, rt : rt + 1])
            scale_chain("B", accs["bmin"][:], accs["bmax"][:], valsB)
            for rt in range(RT):
                p1(a3, rt, accs["amin"][:, rt : rt + 1], accs["amax"][:, rt : rt + 1])
            scale_chain("A", accs["amin"][:], accs["amax"][:], valsA)

            sasb = small.tile([128, 1], f32, tag="sasb", name="sasb")
            nc.vector.tensor_tensor(
                out=sasb[:], in0=valsA[:, 3:4], in1=valsB[:, 3:4], op=OP.mult
            )

            INV_A, CLO_A, CHI_A = valsA[:, 0:1], valsA[:, 1:2], valsA[:, 2:3]
            INV_B, CLO_B, CHI_B = valsB[:, 0:1], valsB[:, 1:2], valsB[:, 2:3]

            # ---- pass 2: re-stream, quantize, transpose ----
            def quantize(src3, rt, inv, clo, chi, qT, colbase, sub_eng):
                t = stage.tile([128, W], f32, tag="stage")
                nc.sync.dma_start(out=t[:], in_=src3[rt])
                u = quant.tile([128, W], f32, tag="quant")
                # u = RNE(x*inv_s + MAGIC), then clip in shifted domain
                nc.scalar.activation(u[:], t[:], ACTF.Copy, bias=MAGIC, scale=inv)
                nc.vector.tensor_scalar(u[:], u[:], chi, clo, OP.min, OP.max)
                qn = qnat_pool.tile([128, W], bf16, tag="qnat")
                if sub_eng == "act":
                    nc.scalar.activation(qn[:], u[:], ACTF.Copy, bias=-MAGIC, scale=1.0)
                else:
                    nc.gpsimd.tensor_scalar_add(qn[:], u[:], -MAGIC)
                nc.sync.dma_start_transpose(
                    out=qT[:, :, colbase : colbase + 128], in_=qn[:]
                )

            # B own half first (scale ready earliest), then A, then B rest:
            # MM group n=0 (B rows 0-511) can start while B rt 4-7 quantize.
            for rt in range(RT):
                quantize(b3, rt, INV_B, CLO_B, CHI_B, qBT, rt * 128,
                         "gpsimd" if rt % 2 else "act")
            for rt in range(RT):
                quantize(a3, rt, INV_A, CLO_A, CHI_A, qAT, rt * 128, "act")
            for rt in range(RT, RT_B):
                quantize(b3, rt, INV_B, CLO_B, CHI_B, qBT, rt * 128,
                         "gpsimd" if rt % 2 else "act")

            # ---- pass 3: matmul + dequant epilogue ----
            for n in range(H // 512):
                for m in range(RT):
                    ps = psum_pool.tile([128, 512], f32)
                    for kt in range(KT):
                        nc.tensor.matmul(
                            ps[:],
                            qAT[:, kt, m * 128 : (m + 1) * 128],
                            qBT[:, kt, n * 512 : (n + 1) * 512],
                            start=(kt == 0),
                            stop=(kt == KT - 1),
                        )
                    o = outp.tile([128, 512], f32, tag="o")
                    nc.vector.tensor_scalar_mul(o[:], ps[:], sasb[:])
                    nc.sync.dma_start(
                        out=out3[m, :, n * 512 : (n + 1) * 512], in_=o[:]
                    )

    nc.compile()
    return nc


def _get_nc():
    if "nc" not in _CACHE:
        _CACHE["nc"] = _build()
    return _CACHE["nc"]


def _in_maps(A, B):
    maps = []
    for c in range(8):
        b, h = c // 2, c % 2
        maps.append(
            {
                "a_own": np.ascontiguousarray(A[b, h * M : (h + 1) * M]),
                "b_rot": np.ascontiguousarray(
                    np.concatenate(
                        [B[b, h * M : (h + 1) * M], B[b, (1 - h) * M : (2 - h) * M]],
                        axis=0,
                    )
                ),
            }
        )
    return maps


def kernel(A: np.ndarray, B: np.ndarray) -> np.ndarray:
    from concourse.bass_utils import run_bass_kernel_spmd

    A = np.ascontiguousarray(A, dtype=np.float32)
    B = np.ascontiguousarray(B, dtype=np.float32)
    nc = _get_nc()

    res = run_bass_kernel_spmd(nc, _in_maps(A, B), core_ids=list(range(8)))
    C = np.empty((BS, H, H), dtype=np.float32)
    for c in range(8):
        b, h = c // 2, c % 2
        o = res.results[c]["out"]  # [512, 1024], columns in rotated order
        C[b, h * M : (h + 1) * M, h * M : (h + 1) * M] = o[:, 0:M]
        C[b, h * M : (h + 1) * M, (1 - h) * M : (2 - h) * M] = o[:, M : 2 * M]
    return C
